# revision 9
# baseline (speedup 1.0000x reference)
"""Trainium2 Bass kernel: multi-head attention with RoPE (causal), 8-core SPMD.

Sharding: 8 cores = 4 batches x 2 head-halves (tensor parallel over heads,
data parallel over batch). Each core computes, for its batch b and its 8
heads: QKV projections, RoPE, causal attention, and a partial output
projection. Host sums the two head-half partials per batch and adds the bias.

All TensorEngine math in bf16 with fp32 PSUM accumulation.
"""

import sys

for _p in ("/opt/trn_rl_repo",):
    if _p not in sys.path:
        sys.path.insert(0, _p)

import numpy as np
import ml_dtypes

import concourse.bass as bass
import concourse.bacc as bacc
import concourse.mybir as mybir
import concourse.tile as tile
from concourse.bass_utils import run_bass_kernel_spmd

BF16 = ml_dtypes.bfloat16

B, T, C = 4, 2048, 2048
H = 16
D = C // H  # 128
ROPE_THETA = 1000000.0
N_CORES = 8
HPC = H // 2          # heads per core (8)
P = 128               # partitions
CHUNK = 512           # moving free dim per matmul
N_CC = C // P         # 16 contraction chunks
N_QC = T // CHUNK     # 4 q-chunks
N_KT = T // P         # 16 k-tiles
N_TT = T // P         # 16 t-tiles
SCALE = 1.0 / float(np.sqrt(D))

_CACHED = {}


def build_kernel():
    """Build the SPMD Bass program (identical on all 8 cores)."""
    fp32 = mybir.dt.float32
    bf16 = mybir.dt.bfloat16

    nc = bacc.Bacc("TRN2", target_bir_lowering=False, debug=False,
                   num_devices=N_CORES)

    # Per-core DRAM inputs (bf16 unless noted)
    xt = nc.dram_tensor("xt", [C, T], bf16, kind="ExternalInput")          # x[b].T
    wq = nc.dram_tensor("wq", [HPC, C, D], bf16, kind="ExternalInput")     # Wq_h.T (perm'd)
    wk = nc.dram_tensor("wk", [HPC, C, D], bf16, kind="ExternalInput")
    wv = nc.dram_tensor("wv", [HPC // 2, C, 2 * D], bf16, kind="ExternalInput")  # head pairs
    wo = nc.dram_tensor("wo", [HPC * D, C], bf16, kind="ExternalInput")    # WoT slice
    cs = nc.dram_tensor("cs", [P, T], bf16, kind="ExternalInput")          # [cosT;sinT]
    sc = nc.dram_tensor("sc", [P, T], bf16, kind="ExternalInput")          # [sinT;cosT]
    tri = nc.dram_tensor("tri", [P, P], bf16, kind="ExternalInput")        # k<=q mask
    y = nc.dram_tensor("y", [T, C], fp32, kind="ExternalOutput")

    Exp = mybir.ActivationFunctionType.Exp

    with tile.TileContext(nc) as tc:
        with (
            tc.tile_pool(name="const", bufs=1) as const_pool,
            tc.tile_pool(name="xtp", bufs=1) as xt_pool,
            tc.tile_pool(name="wqk", bufs=4) as wqk_pool,
            tc.tile_pool(name="wvp", bufs=2) as wv_pool,
            tc.tile_pool(name="vw", bufs=4) as vw_pool,
            tc.tile_pool(name="swp", bufs=4) as swp_pool,
            tc.tile_pool(name="qktr", bufs=2) as qk_pool,
            tc.tile_pool(name="vsb", bufs=2) as v_pool,
            tc.tile_pool(name="pt", bufs=3) as pt_pool,
            tc.tile_pool(name="rcp", bufs=2) as rcp_pool,
            tc.tile_pool(name="aot", bufs=1) as aot_pool,
            tc.tile_pool(name="wop", bufs=16) as wo_pool,
            tc.tile_pool(name="yst", bufs=3) as y_pool,
            tc.tile_pool(name="ph1", bufs=2, space="PSUM") as ph1_psum,
            tc.tile_pool(name="pst", bufs=2, space="PSUM") as s_psum,
            tc.tile_pool(name="po", bufs=2, space="PSUM") as o_psum,
            tc.tile_pool(name="pr", bufs=2, space="PSUM") as r_psum,
        ):
            # ---- weight prefetch (head 0 first, before the big xt load) ----
            qk_w = {}

            def load_qk_w(h):
                wq_sb = wqk_pool.tile([P, N_CC, D], bf16, tag="wqk")
                nc.sync.dma_start(
                    out=wq_sb, in_=wq.ap()[h].rearrange("(cc p) d -> p cc d", p=P))
                wk_sb = wqk_pool.tile([P, N_CC, D], bf16, tag="wqk")
                nc.sync.dma_start(
                    out=wk_sb, in_=wk.ap()[h].rearrange("(cc p) d -> p cc d", p=P))
                qk_w[h] = (wq_sb, wk_sb)

            wv_w = {}

            def load_wv(pair):
                w_sb = wv_pool.tile([P, N_CC, 2 * D], bf16, tag="wv")
                nc.sync.dma_start(
                    out=w_sb, in_=wv.ap()[pair].rearrange("(cc p) d -> p cc d", p=P))
                wv_w[pair] = w_sb

            load_qk_w(0)

            # ---- constants / big resident tensors ----
            cs_sb = const_pool.tile([P, T], bf16)
            sc_sb = const_pool.tile([P, T], bf16)
            tri_sb = const_pool.tile([P, P], bf16)
            ones_sb = const_pool.tile([P, P], bf16)
            nc.sync.dma_start(out=cs_sb, in_=cs.ap())
            nc.sync.dma_start(out=sc_sb, in_=sc.ap())
            nc.sync.dma_start(out=tri_sb, in_=tri.ap())
            nc.vector.memset(ones_sb, 1.0)

            xt_sb = xt_pool.tile([P, N_CC, T], bf16)
            xt_r = xt.ap().rearrange("(cc p) t -> p cc t", p=P)
            for cc in range(N_CC):
                eng = nc.sync if cc % 2 == 0 else nc.gpsimd
                eng.dma_start(out=xt_sb[:, cc, :], in_=xt_r[:, cc, :])
            load_wv(0)

            aot_sb = aot_pool.tile([P, HPC, T], bf16)  # attn-out^T, all heads

            def project_qk(w_sb, name):
                """Q^T (or K^T) with RoPE -> bf16 [P, T] tile."""
                out_sb = qk_pool.tile([P, T], bf16, tag=f"{name}tr")
                for qc in range(N_QC):
                    ps = ph1_psum.tile([P, CHUNK], fp32, tag="ph1")
                    for cc in range(N_CC):
                        nc.tensor.matmul(
                            ps, lhsT=w_sb[:, cc, :],
                            rhs=xt_sb[:, cc, qc * CHUNK:(qc + 1) * CHUNK],
                            start=(cc == 0), stop=(cc == N_CC - 1))
                    # RoPE: rows 0:64 = even dims (e), 64:128 = odd dims (o)
                    # re = e*cos - o*sin ; ro = o*cos + e*sin
                    v_t = vw_pool.tile([P, CHUNK], bf16, tag="vw")   # [e*cos; o*sin]
                    w_t = vw_pool.tile([P, CHUNK], bf16, tag="vw")   # [e*sin; o*cos]
                    nc.vector.tensor_mul(v_t, ps, cs_sb[:, qc * CHUNK:(qc + 1) * CHUNK])
                    nc.vector.tensor_mul(w_t, ps, sc_sb[:, qc * CHUNK:(qc + 1) * CHUNK])
                    sw = swp_pool.tile([P, CHUNK], bf16, tag="swp")
                    # sw[0:64] <- o*sin (from rows 64:128); sw[64:128] <- e*sin (rows 0:64)
                    nc.gpsimd.dma_start(out=sw[0:64, :], in_=v_t[64:128, :])
                    nc.gpsimd.dma_start(out=sw[64:128, :], in_=w_t[0:64, :])
                    sl = slice(qc * CHUNK, (qc + 1) * CHUNK)
                    nc.gpsimd.tensor_sub(out_sb[0:64, sl], v_t[0:64, :], sw[0:64, :])
                    nc.gpsimd.tensor_add(out_sb[64:128, sl], w_t[64:128, :], sw[64:128, :])
                return out_sb

            def project_v(pair):
                """V for head pair -> bf16 [P, N_KT, 2D] tile (t-partition)."""
                w_sb = wv_w.pop(pair)
                v_sb = v_pool.tile([P, N_KT, 2 * D], bf16, tag="vsb")
                for tt in range(N_TT):
                    ps = ph1_psum.tile([P, CHUNK], fp32, tag="ph1")
                    psv = ps[:, 0:2 * D]
                    for cc in range(N_CC):
                        nc.tensor.matmul(
                            psv, lhsT=xt_sb[:, cc, tt * P:(tt + 1) * P],
                            rhs=w_sb[:, cc, :],
                            start=(cc == 0), stop=(cc == N_CC - 1))
                    nc.scalar.copy(v_sb[:, tt, :], psv)
                return v_sb

            def attention(h, qtr, ktr, v_sb, v_col):
                """Causal attention for head h -> writes aot_sb[:, h, :]."""
                for qc in range(N_QC):
                    q0 = qc * CHUNK
                    ps_o = o_psum.tile([P, CHUNK], fp32, tag="po")
                    ps_r = r_psum.tile([P, CHUNK], fp32, tag="pr")
                    n_kt = 4 * qc + 4
                    for kt in range(n_kt):
                        j = kt - 4 * qc  # >= 0 on diagonal tiles
                        n0 = max(0, j * P)  # trimmed col start within chunk
                        w = CHUNK - n0
                        ps_s = s_psum.tile([P, CHUNK], fp32, tag="pst")
                        nc.tensor.matmul(
                            ps_s[:, 0:w], lhsT=ktr[:, kt * P:(kt + 1) * P],
                            rhs=qtr[:, q0 + n0:q0 + CHUNK], start=True, stop=True)
                        pt = pt_pool.tile([P, CHUNK], bf16, tag="pt")
                        nc.scalar.activation(pt[:, 0:w], ps_s[:, 0:w], Exp, scale=SCALE)
                        if j >= 0:
                            # triangle mask on first P cols of the trimmed range
                            nc.gpsimd.tensor_mul(pt[:, 0:P], pt[:, 0:P], tri_sb)
                        nc.tensor.matmul(
                            ps_o[:, n0:CHUNK],
                            lhsT=v_sb[:, kt, v_col * D:(v_col + 1) * D],
                            rhs=pt[:, 0:w], start=(kt == 0), stop=(kt == n_kt - 1))
                        nc.tensor.matmul(
                            ps_r[:, n0:CHUNK], lhsT=ones_sb, rhs=pt[:, 0:w],
                            start=(kt == 0), stop=(kt == n_kt - 1))
                    rec = rcp_pool.tile([P, CHUNK], fp32, tag="rcp")
                    nc.vector.reciprocal_approx_fast(out=rec, in_=ps_r)
                    nc.vector.tensor_mul(aot_sb[:, h, q0:q0 + CHUNK], ps_o, rec)

            # ---- phase 1 + 2, software-pipelined one head ahead ----
            v_tiles = {}
            qk_tiles = {}

            def produce(h):
                wq_sb, wk_sb = qk_w.pop(h)
                qtr = project_qk(wq_sb, "wq")
                ktr = project_qk(wk_sb, "wk")
                qk_tiles[h] = (qtr, ktr)
                if h % 2 == 0:
                    v_tiles[h // 2] = project_v(h // 2)

            produce(0)
            for h in range(HPC):
                nxt = h + 1
                if nxt < HPC:
                    load_qk_w(nxt)
                    if nxt % 2 == 0:
                        load_wv(nxt // 2)
                    produce(nxt)
                qtr, ktr = qk_tiles.pop(h)
                attention(h, qtr, ktr, v_tiles[h // 2], h % 2)

            # ---- phase 3: partial output projection y = AOT^T @ WoT ----
            for co in range(N_QC):
                wo_tiles = []
                for cb in range(HPC):  # contraction chunks == heads
                    w_sb = wo_pool.tile([P, CHUNK], bf16, tag="wo")
                    nc.sync.dma_start(
                        out=w_sb,
                        in_=wo.ap()[cb * P:(cb + 1) * P,
                                    co * CHUNK:(co + 1) * CHUNK])
                    wo_tiles.append(w_sb)
                for tt in range(N_TT):
                    ps = ph1_psum.tile([P, CHUNK], fp32, tag="ph1")
                    for cb in range(HPC):
                        nc.tensor.matmul(
                            ps, lhsT=aot_sb[:, cb, tt * P:(tt + 1) * P],
                            rhs=wo_tiles[cb],
                            start=(cb == 0), stop=(cb == HPC - 1))
                    yt = y_pool.tile([P, CHUNK], fp32, tag="y")
                    nc.scalar.copy(yt, ps)
                    nc.scalar.dma_start(
                        out=y.ap()[tt * P:(tt + 1) * P, co * CHUNK:(co + 1) * CHUNK],
                        in_=yt)

    nc.finalize()
    return nc


def _host_prep(x, Wq, Wk, Wv, Wo):
    """Build the 8 per-core input maps."""
    perm = np.concatenate([np.arange(0, D, 2), np.arange(1, D, 2)])

    inv_freq = 1.0 / ROPE_THETA ** (np.arange(0, D, 2, dtype=np.float32) / D)
    pos = np.arange(T, dtype=np.float32)
    freqs = np.einsum("i,j->ij", pos, inv_freq)  # [T, 64]
    cosT = np.cos(freqs).T.astype(np.float32)
    sinT = np.sin(freqs).T.astype(np.float32)
    cs = np.concatenate([cosT, sinT], 0).astype(BF16)
    sc = np.concatenate([sinT, cosT], 0).astype(BF16)
    tri = (np.arange(P)[:, None] <= np.arange(P)[None, :]).astype(BF16)

    halves = []
    for g in range(2):
        heads = range(g * HPC, (g + 1) * HPC)
        wq_g = np.stack([Wq[h * D:(h + 1) * D, :][perm, :].T.astype(BF16)
                         for h in heads])                       # [8, C, D]
        wk_g = np.stack([Wk[h * D:(h + 1) * D, :][perm, :].T.astype(BF16)
                         for h in heads])
        wv_g = np.stack([Wv[(g * HPC + 2 * p) * D:(g * HPC + 2 * p + 2) * D, :]
                         .T.astype(BF16) for p in range(HPC // 2)])  # [4, C, 2D]
        wo_g = Wo[:, g * HPC * D:(g + 1) * HPC * D].T.astype(BF16)   # [1024, C]
        halves.append(dict(wq=wq_g, wk=wk_g, wv=wv_g, wo=wo_g))

    in_maps = []
    for core in range(N_CORES):
        b, g = core // 2, core % 2
        m = dict(halves[g])
        m["xt"] = np.ascontiguousarray(x[b].T).astype(BF16)
        m["cs"] = cs
        m["sc"] = sc
        m["tri"] = tri
        in_maps.append(m)
    return in_maps


def kernel(x, Wq, Wk, Wv, Wo, bo):
    x = np.asarray(x, dtype=np.float32)
    Wq = np.asarray(Wq, dtype=np.float32)
    Wk = np.asarray(Wk, dtype=np.float32)
    Wv = np.asarray(Wv, dtype=np.float32)
    Wo = np.asarray(Wo, dtype=np.float32)
    bo = np.asarray(bo, dtype=np.float32)

    if "nc" not in _CACHED:
        _CACHED["nc"] = build_kernel()
    nc = _CACHED["nc"]

    in_maps = _host_prep(x, Wq, Wk, Wv, Wo)
    res = run_bass_kernel_spmd(nc, in_maps, core_ids=list(range(N_CORES)))

    out = np.empty((B, T, C), np.float32)
    for b in range(B):
        out[b] = res.results[2 * b]["y"] + res.results[2 * b + 1]["y"] + bo
    return out


# revision 10
# speedup vs baseline: 1.0514x; 1.0514x over previous
"""Trainium2 Bass kernel: multi-head attention with RoPE (causal), 8-core SPMD.

Sharding: 8 cores = 4 batches x 2 head-halves (tensor parallel over heads,
data parallel over batch). Each core computes, for its batch b and its 8
heads: QKV projections, RoPE, causal attention, and a partial output
projection. Host sums the two head-half partials per batch and adds the bias.

All TensorEngine math in bf16 with fp32 PSUM accumulation.
"""

import sys

for _p in ("/opt/trn_rl_repo",):
    if _p not in sys.path:
        sys.path.insert(0, _p)

import numpy as np
import ml_dtypes

import concourse.bass as bass
import concourse.bacc as bacc
import concourse.mybir as mybir
import concourse.tile as tile
from concourse.bass_utils import run_bass_kernel_spmd

BF16 = ml_dtypes.bfloat16

B, T, C = 4, 2048, 2048
H = 16
D = C // H  # 128
ROPE_THETA = 1000000.0
N_CORES = 8
HPC = H // 2          # heads per core (8)
P = 128               # partitions
CHUNK = 512           # moving free dim per matmul
N_CC = C // P         # 16 contraction chunks
N_QC = T // CHUNK     # 4 q-chunks
N_KT = T // P         # 16 k-tiles
N_TT = T // P         # 16 t-tiles
SCALE = 1.0 / float(np.sqrt(D))

_CACHED = {}


def build_kernel():
    """Build the SPMD Bass program (identical on all 8 cores)."""
    fp32 = mybir.dt.float32
    bf16 = mybir.dt.bfloat16

    nc = bacc.Bacc("TRN2", target_bir_lowering=False, debug=False,
                   num_devices=N_CORES)

    # Per-core DRAM inputs (bf16 unless noted)
    xt = nc.dram_tensor("xt", [C, T], bf16, kind="ExternalInput")          # x[b].T
    wq = nc.dram_tensor("wq", [HPC, C, D], bf16, kind="ExternalInput")     # Wq_h.T (perm'd)
    wk = nc.dram_tensor("wk", [HPC, C, D], bf16, kind="ExternalInput")
    wv = nc.dram_tensor("wv", [HPC // 2, C, 2 * D], bf16, kind="ExternalInput")  # head pairs
    wo = nc.dram_tensor("wo", [HPC * D, C], bf16, kind="ExternalInput")    # WoT slice
    cs = nc.dram_tensor("cs", [P, T], bf16, kind="ExternalInput")          # [cosT;sinT]
    sc = nc.dram_tensor("sc", [P, T], bf16, kind="ExternalInput")          # [sinT;cosT]
    tri = nc.dram_tensor("tri", [P, P], bf16, kind="ExternalInput")        # k<=q mask
    y = nc.dram_tensor("y", [T, C], fp32, kind="ExternalOutput")

    Exp = mybir.ActivationFunctionType.Exp

    with tile.TileContext(nc) as tc:
        with (
            tc.tile_pool(name="const", bufs=1) as const_pool,
            tc.tile_pool(name="xtp", bufs=1) as xt_pool,
            tc.tile_pool(name="wqk", bufs=4) as wqk_pool,
            tc.tile_pool(name="wvp", bufs=2) as wv_pool,
            tc.tile_pool(name="vw", bufs=4) as vw_pool,
            tc.tile_pool(name="swp", bufs=4) as swp_pool,
            tc.tile_pool(name="qktr", bufs=2) as qk_pool,
            tc.tile_pool(name="vsb", bufs=2) as v_pool,
            tc.tile_pool(name="pt", bufs=3) as pt_pool,
            tc.tile_pool(name="rcp", bufs=2) as rcp_pool,
            tc.tile_pool(name="aot", bufs=1) as aot_pool,
            tc.tile_pool(name="wop", bufs=16) as wo_pool,
            tc.tile_pool(name="yst", bufs=3) as y_pool,
            tc.tile_pool(name="ph1", bufs=2, space="PSUM") as ph1_psum,
            tc.tile_pool(name="pst", bufs=2, space="PSUM") as s_psum,
            tc.tile_pool(name="po", bufs=2, space="PSUM") as o_psum,
            tc.tile_pool(name="pr", bufs=2, space="PSUM") as r_psum,
        ):
            # ---- weight prefetch (head 0 first, before the big xt load) ----
            qk_w = {}

            def load_qk_w(h):
                wq_sb = wqk_pool.tile([P, N_CC, D], bf16, tag="wqk")
                nc.sync.dma_start(
                    out=wq_sb, in_=wq.ap()[h].rearrange("(cc p) d -> p cc d", p=P))
                wk_sb = wqk_pool.tile([P, N_CC, D], bf16, tag="wqk")
                nc.sync.dma_start(
                    out=wk_sb, in_=wk.ap()[h].rearrange("(cc p) d -> p cc d", p=P))
                qk_w[h] = (wq_sb, wk_sb)

            wv_w = {}

            def load_wv(pair):
                w_sb = wv_pool.tile([P, N_CC, 2 * D], bf16, tag="wv")
                nc.sync.dma_start(
                    out=w_sb, in_=wv.ap()[pair].rearrange("(cc p) d -> p cc d", p=P))
                wv_w[pair] = w_sb

            load_qk_w(0)

            # ---- constants / big resident tensors ----
            cs_sb = const_pool.tile([P, T], bf16)
            sc_sb = const_pool.tile([P, T], bf16)
            tri_sb = const_pool.tile([P, P], bf16)
            ones_sb = const_pool.tile([P, P], bf16)
            nc.sync.dma_start(out=cs_sb, in_=cs.ap())
            nc.sync.dma_start(out=sc_sb, in_=sc.ap())
            nc.sync.dma_start(out=tri_sb, in_=tri.ap())
            nc.vector.memset(ones_sb, 1.0)

            xt_sb = xt_pool.tile([P, N_CC, T], bf16)
            xt_r = xt.ap().rearrange("(cc p) t -> p cc t", p=P)
            for cc in range(N_CC):
                eng = nc.sync if cc % 2 == 0 else nc.gpsimd
                eng.dma_start(out=xt_sb[:, cc, :], in_=xt_r[:, cc, :])
            load_wv(0)

            aot_sb = aot_pool.tile([P, HPC, T], bf16)  # attn-out^T, all heads

            def project_qk(w_sb, name):
                """Q^T (or K^T) with RoPE -> bf16 [P, T] tile."""
                out_sb = qk_pool.tile([P, T], bf16, tag=f"{name}tr")
                for qc in range(N_QC):
                    ps = ph1_psum.tile([P, CHUNK], fp32, tag="ph1")
                    for cc in range(N_CC):
                        nc.tensor.matmul(
                            ps, lhsT=w_sb[:, cc, :],
                            rhs=xt_sb[:, cc, qc * CHUNK:(qc + 1) * CHUNK],
                            start=(cc == 0), stop=(cc == N_CC - 1))
                    # RoPE: rows 0:64 = even dims (e), 64:128 = odd dims (o)
                    # re = e*cos - o*sin ; ro = o*cos + e*sin
                    v_t = vw_pool.tile([P, CHUNK], bf16, tag="vw")   # [e*cos; o*sin]
                    w_t = vw_pool.tile([P, CHUNK], bf16, tag="vw")   # [e*sin; o*cos]
                    nc.vector.tensor_mul(v_t, ps, cs_sb[:, qc * CHUNK:(qc + 1) * CHUNK])
                    nc.vector.tensor_mul(w_t, ps, sc_sb[:, qc * CHUNK:(qc + 1) * CHUNK])
                    sw = swp_pool.tile([P, CHUNK], bf16, tag="swp")
                    # sw[0:64] <- o*sin (from rows 64:128); sw[64:128] <- e*sin (rows 0:64)
                    nc.gpsimd.dma_start(out=sw[0:64, :], in_=v_t[64:128, :])
                    nc.gpsimd.dma_start(out=sw[64:128, :], in_=w_t[0:64, :])
                    sl = slice(qc * CHUNK, (qc + 1) * CHUNK)
                    nc.gpsimd.tensor_sub(out_sb[0:64, sl], v_t[0:64, :], sw[0:64, :])
                    nc.gpsimd.tensor_add(out_sb[64:128, sl], w_t[64:128, :], sw[64:128, :])
                return out_sb

            def project_v(pair):
                """V for head pair -> bf16 [P, N_KT, 2D] tile (t-partition)."""
                w_sb = wv_w.pop(pair)
                v_sb = v_pool.tile([P, N_KT, 2 * D], bf16, tag="vsb")
                for tt in range(N_TT):
                    ps = ph1_psum.tile([P, CHUNK], fp32, tag="ph1")
                    psv = ps[:, 0:2 * D]
                    for cc in range(N_CC):
                        nc.tensor.matmul(
                            psv, lhsT=xt_sb[:, cc, tt * P:(tt + 1) * P],
                            rhs=w_sb[:, cc, :],
                            start=(cc == 0), stop=(cc == N_CC - 1))
                    nc.scalar.copy(v_sb[:, tt, :], psv)
                return v_sb

            def attention(h, qtr, ktr, v_sb, v_col):
                """Causal attention for head h -> writes aot_sb[:, h, :]."""
                for qc in range(N_QC):
                    q0 = qc * CHUNK
                    ps_o = o_psum.tile([P, CHUNK], fp32, tag="po")
                    ps_r = r_psum.tile([P, CHUNK], fp32, tag="pr")
                    n_kt = 4 * qc + 4
                    for kt in range(n_kt):
                        j = kt - 4 * qc  # >= 0 on diagonal tiles
                        n0 = max(0, j * P)  # trimmed col start within chunk
                        w = CHUNK - n0
                        ps_s = s_psum.tile([P, CHUNK], fp32, tag="pst")
                        nc.tensor.matmul(
                            ps_s[:, 0:w], lhsT=ktr[:, kt * P:(kt + 1) * P],
                            rhs=qtr[:, q0 + n0:q0 + CHUNK], start=True, stop=True)
                        pt = pt_pool.tile([P, CHUNK], bf16, tag="pt")
                        nc.scalar.activation(pt[:, 0:w], ps_s[:, 0:w], Exp, scale=SCALE)
                        if j >= 0:
                            # triangle mask on first P cols of the trimmed range
                            nc.vector.tensor_mul(pt[:, 0:P], pt[:, 0:P], tri_sb)
                        nc.tensor.matmul(
                            ps_o[:, n0:CHUNK],
                            lhsT=v_sb[:, kt, v_col * D:(v_col + 1) * D],
                            rhs=pt[:, 0:w], start=(kt == 0), stop=(kt == n_kt - 1))
                        nc.tensor.matmul(
                            ps_r[:, n0:CHUNK], lhsT=ones_sb, rhs=pt[:, 0:w],
                            start=(kt == 0), stop=(kt == n_kt - 1))
                    rec = rcp_pool.tile([P, CHUNK], fp32, tag="rcp")
                    nc.vector.reciprocal_approx_fast(out=rec, in_=ps_r)
                    nc.vector.tensor_mul(aot_sb[:, h, q0:q0 + CHUNK], ps_o, rec)

            # ---- phase 1 + 2, software-pipelined one head ahead ----
            v_tiles = {}
            qk_tiles = {}

            def produce(h):
                wq_sb, wk_sb = qk_w.pop(h)
                qtr = project_qk(wq_sb, "wq")
                ktr = project_qk(wk_sb, "wk")
                qk_tiles[h] = (qtr, ktr)
                if h % 2 == 0:
                    v_tiles[h // 2] = project_v(h // 2)

            produce(0)
            for h in range(HPC):
                nxt = h + 1
                if nxt < HPC:
                    load_qk_w(nxt)
                    if nxt % 2 == 0:
                        load_wv(nxt // 2)
                    produce(nxt)
                qtr, ktr = qk_tiles.pop(h)
                attention(h, qtr, ktr, v_tiles[h // 2], h % 2)

            # ---- phase 3: partial output projection y = AOT^T @ WoT ----
            for co in range(N_QC):
                wo_tiles = []
                for cb in range(HPC):  # contraction chunks == heads
                    w_sb = wo_pool.tile([P, CHUNK], bf16, tag="wo")
                    nc.sync.dma_start(
                        out=w_sb,
                        in_=wo.ap()[cb * P:(cb + 1) * P,
                                    co * CHUNK:(co + 1) * CHUNK])
                    wo_tiles.append(w_sb)
                for tt in range(N_TT):
                    ps = ph1_psum.tile([P, CHUNK], fp32, tag="ph1")
                    for cb in range(HPC):
                        nc.tensor.matmul(
                            ps, lhsT=aot_sb[:, cb, tt * P:(tt + 1) * P],
                            rhs=wo_tiles[cb],
                            start=(cb == 0), stop=(cb == HPC - 1))
                    yt = y_pool.tile([P, CHUNK], fp32, tag="y")
                    nc.scalar.copy(yt, ps)
                    nc.scalar.dma_start(
                        out=y.ap()[tt * P:(tt + 1) * P, co * CHUNK:(co + 1) * CHUNK],
                        in_=yt)

    nc.finalize()
    return nc


def _host_prep(x, Wq, Wk, Wv, Wo):
    """Build the 8 per-core input maps."""
    perm = np.concatenate([np.arange(0, D, 2), np.arange(1, D, 2)])

    inv_freq = 1.0 / ROPE_THETA ** (np.arange(0, D, 2, dtype=np.float32) / D)
    pos = np.arange(T, dtype=np.float32)
    freqs = np.einsum("i,j->ij", pos, inv_freq)  # [T, 64]
    cosT = np.cos(freqs).T.astype(np.float32)
    sinT = np.sin(freqs).T.astype(np.float32)
    cs = np.concatenate([cosT, sinT], 0).astype(BF16)
    sc = np.concatenate([sinT, cosT], 0).astype(BF16)
    tri = (np.arange(P)[:, None] <= np.arange(P)[None, :]).astype(BF16)

    halves = []
    for g in range(2):
        heads = range(g * HPC, (g + 1) * HPC)
        wq_g = np.stack([Wq[h * D:(h + 1) * D, :][perm, :].T.astype(BF16)
                         for h in heads])                       # [8, C, D]
        wk_g = np.stack([Wk[h * D:(h + 1) * D, :][perm, :].T.astype(BF16)
                         for h in heads])
        wv_g = np.stack([Wv[(g * HPC + 2 * p) * D:(g * HPC + 2 * p + 2) * D, :]
                         .T.astype(BF16) for p in range(HPC // 2)])  # [4, C, 2D]
        wo_g = Wo[:, g * HPC * D:(g + 1) * HPC * D].T.astype(BF16)   # [1024, C]
        halves.append(dict(wq=wq_g, wk=wk_g, wv=wv_g, wo=wo_g))

    in_maps = []
    for core in range(N_CORES):
        b, g = core // 2, core % 2
        m = dict(halves[g])
        m["xt"] = np.ascontiguousarray(x[b].T).astype(BF16)
        m["cs"] = cs
        m["sc"] = sc
        m["tri"] = tri
        in_maps.append(m)
    return in_maps


def kernel(x, Wq, Wk, Wv, Wo, bo):
    x = np.asarray(x, dtype=np.float32)
    Wq = np.asarray(Wq, dtype=np.float32)
    Wk = np.asarray(Wk, dtype=np.float32)
    Wv = np.asarray(Wv, dtype=np.float32)
    Wo = np.asarray(Wo, dtype=np.float32)
    bo = np.asarray(bo, dtype=np.float32)

    if "nc" not in _CACHED:
        _CACHED["nc"] = build_kernel()
    nc = _CACHED["nc"]

    in_maps = _host_prep(x, Wq, Wk, Wv, Wo)
    res = run_bass_kernel_spmd(nc, in_maps, core_ids=list(range(N_CORES)))

    out = np.empty((B, T, C), np.float32)
    for b in range(B):
        out[b] = res.results[2 * b]["y"] + res.results[2 * b + 1]["y"] + bo
    return out


# revision 11
# speedup vs baseline: 1.0549x; 1.0033x over previous
"""Trainium2 Bass kernel: multi-head attention with RoPE (causal), 8-core SPMD.

Sharding: 8 cores = 4 batches x 2 head-halves (tensor parallel over heads,
data parallel over batch). Each core computes, for its batch b and its 8
heads: QKV projections, RoPE, causal attention, and a partial output
projection. Host sums the two head-half partials per batch and adds the bias.

All TensorEngine math in bf16 with fp32 PSUM accumulation.
"""

import sys

for _p in ("/opt/trn_rl_repo",):
    if _p not in sys.path:
        sys.path.insert(0, _p)

import numpy as np
import ml_dtypes

import concourse.bass as bass
import concourse.bacc as bacc
import concourse.mybir as mybir
import concourse.tile as tile
from concourse.bass_utils import run_bass_kernel_spmd

BF16 = ml_dtypes.bfloat16

B, T, C = 4, 2048, 2048
H = 16
D = C // H  # 128
ROPE_THETA = 1000000.0
N_CORES = 8
HPC = H // 2          # heads per core (8)
P = 128               # partitions
CHUNK = 512           # moving free dim per matmul
N_CC = C // P         # 16 contraction chunks
N_QC = T // CHUNK     # 4 q-chunks
N_KT = T // P         # 16 k-tiles
N_TT = T // P         # 16 t-tiles
SCALE = 1.0 / float(np.sqrt(D))

_CACHED = {}


def build_kernel():
    """Build the SPMD Bass program (identical on all 8 cores)."""
    fp32 = mybir.dt.float32
    bf16 = mybir.dt.bfloat16

    nc = bacc.Bacc("TRN2", target_bir_lowering=False, debug=False,
                   num_devices=N_CORES)

    # Per-core DRAM inputs (bf16 unless noted)
    xt = nc.dram_tensor("xt", [C, T], bf16, kind="ExternalInput")          # x[b].T
    wq = nc.dram_tensor("wq", [HPC, C, D], bf16, kind="ExternalInput")     # Wq_h.T (perm'd)
    wk = nc.dram_tensor("wk", [HPC, C, D], bf16, kind="ExternalInput")
    wv = nc.dram_tensor("wv", [HPC // 2, C, 2 * D], bf16, kind="ExternalInput")  # head pairs
    wo = nc.dram_tensor("wo", [HPC * D, C], bf16, kind="ExternalInput")    # WoT slice
    cs = nc.dram_tensor("cs", [P, T], bf16, kind="ExternalInput")          # [cosT;sinT]
    sc = nc.dram_tensor("sc", [P, T], bf16, kind="ExternalInput")          # [sinT;cosT]
    tri = nc.dram_tensor("tri", [P, P], bf16, kind="ExternalInput")        # k<=q mask
    y = nc.dram_tensor("y", [T, C], fp32, kind="ExternalOutput")

    Exp = mybir.ActivationFunctionType.Exp

    with tile.TileContext(nc) as tc:
        with (
            tc.tile_pool(name="const", bufs=1) as const_pool,
            tc.tile_pool(name="xtp", bufs=1) as xt_pool,
            tc.tile_pool(name="wqk", bufs=4) as wqk_pool,
            tc.tile_pool(name="wvp", bufs=2) as wv_pool,
            tc.tile_pool(name="vw", bufs=4) as vw_pool,
            tc.tile_pool(name="swp", bufs=4) as swp_pool,
            tc.tile_pool(name="qktr", bufs=2) as qk_pool,
            tc.tile_pool(name="vsb", bufs=2) as v_pool,
            tc.tile_pool(name="pt", bufs=3) as pt_pool,
            tc.tile_pool(name="rcp", bufs=2) as rcp_pool,
            tc.tile_pool(name="aot", bufs=1) as aot_pool,
            tc.tile_pool(name="wop", bufs=16) as wo_pool,
            tc.tile_pool(name="yst", bufs=3) as y_pool,
            tc.tile_pool(name="ph1", bufs=2, space="PSUM") as ph1_psum,
            tc.tile_pool(name="pst", bufs=2, space="PSUM") as s_psum,
            tc.tile_pool(name="po", bufs=2, space="PSUM") as o_psum,
            tc.tile_pool(name="pr", bufs=2, space="PSUM") as r_psum,
        ):
            # ---- weight prefetch (head 0 first, before the big xt load) ----
            qk_w = {}

            def load_qk_w(h):
                wq_sb = wqk_pool.tile([P, N_CC, D], bf16, tag="wqk")
                nc.sync.dma_start(
                    out=wq_sb, in_=wq.ap()[h].rearrange("(cc p) d -> p cc d", p=P))
                wk_sb = wqk_pool.tile([P, N_CC, D], bf16, tag="wqk")
                nc.sync.dma_start(
                    out=wk_sb, in_=wk.ap()[h].rearrange("(cc p) d -> p cc d", p=P))
                qk_w[h] = (wq_sb, wk_sb)

            wv_w = {}

            def load_wv(pair):
                w_sb = wv_pool.tile([P, N_CC, 2 * D], bf16, tag="wv")
                nc.sync.dma_start(
                    out=w_sb, in_=wv.ap()[pair].rearrange("(cc p) d -> p cc d", p=P))
                wv_w[pair] = w_sb

            load_qk_w(0)

            # ---- constants / big resident tensors ----
            cs_sb = const_pool.tile([P, T], bf16)
            sc_sb = const_pool.tile([P, T], bf16)
            tri_sb = const_pool.tile([P, P], bf16)
            ones_sb = const_pool.tile([P, P], bf16)
            nc.sync.dma_start(out=cs_sb, in_=cs.ap())
            nc.sync.dma_start(out=sc_sb, in_=sc.ap())
            nc.sync.dma_start(out=tri_sb, in_=tri.ap())
            nc.vector.memset(ones_sb, 1.0)

            xt_sb = xt_pool.tile([P, N_CC, T], bf16)
            xt_r = xt.ap().rearrange("(cc p) t -> p cc t", p=P)
            for qc in range(N_QC):
                for cc in range(N_CC):
                    eng = nc.sync if cc % 2 == 0 else nc.gpsimd
                    eng.dma_start(
                        out=xt_sb[:, cc, qc * CHUNK:(qc + 1) * CHUNK],
                        in_=xt_r[:, cc, qc * CHUNK:(qc + 1) * CHUNK])
            load_wv(0)

            aot_sb = aot_pool.tile([P, HPC, T], bf16)  # attn-out^T, all heads

            def project_qk(w_sb, name):
                """Q^T (or K^T) with RoPE -> bf16 [P, T] tile."""
                out_sb = qk_pool.tile([P, T], bf16, tag=f"{name}tr")
                for qc in range(N_QC):
                    ps = ph1_psum.tile([P, CHUNK], fp32, tag="ph1")
                    for cc in range(N_CC):
                        nc.tensor.matmul(
                            ps, lhsT=w_sb[:, cc, :],
                            rhs=xt_sb[:, cc, qc * CHUNK:(qc + 1) * CHUNK],
                            start=(cc == 0), stop=(cc == N_CC - 1))
                    # RoPE: rows 0:64 = even dims (e), 64:128 = odd dims (o)
                    # re = e*cos - o*sin ; ro = o*cos + e*sin
                    v_t = vw_pool.tile([P, CHUNK], bf16, tag="vw")   # [e*cos; o*sin]
                    w_t = vw_pool.tile([P, CHUNK], bf16, tag="vw")   # [e*sin; o*cos]
                    nc.vector.tensor_mul(v_t, ps, cs_sb[:, qc * CHUNK:(qc + 1) * CHUNK])
                    nc.vector.tensor_mul(w_t, ps, sc_sb[:, qc * CHUNK:(qc + 1) * CHUNK])
                    sw = swp_pool.tile([P, CHUNK], bf16, tag="swp")
                    # sw[0:64] <- o*sin (from rows 64:128); sw[64:128] <- e*sin (rows 0:64)
                    nc.gpsimd.dma_start(out=sw[0:64, :], in_=v_t[64:128, :])
                    nc.gpsimd.dma_start(out=sw[64:128, :], in_=w_t[0:64, :])
                    sl = slice(qc * CHUNK, (qc + 1) * CHUNK)
                    nc.gpsimd.tensor_sub(out_sb[0:64, sl], v_t[0:64, :], sw[0:64, :])
                    nc.gpsimd.tensor_add(out_sb[64:128, sl], w_t[64:128, :], sw[64:128, :])
                return out_sb

            def project_v(pair):
                """V for head pair -> bf16 [P, N_KT, 2D] tile (t-partition)."""
                w_sb = wv_w.pop(pair)
                v_sb = v_pool.tile([P, N_KT, 2 * D], bf16, tag="vsb")
                for tt in range(N_TT):
                    ps = ph1_psum.tile([P, CHUNK], fp32, tag="ph1")
                    psv = ps[:, 0:2 * D]
                    for cc in range(N_CC):
                        nc.tensor.matmul(
                            psv, lhsT=xt_sb[:, cc, tt * P:(tt + 1) * P],
                            rhs=w_sb[:, cc, :],
                            start=(cc == 0), stop=(cc == N_CC - 1))
                    nc.scalar.copy(v_sb[:, tt, :], psv)
                return v_sb

            def attention(h, qtr, ktr, v_sb, v_col):
                """Causal attention for head h -> writes aot_sb[:, h, :]."""
                for qc in range(N_QC):
                    q0 = qc * CHUNK
                    ps_o = o_psum.tile([P, CHUNK], fp32, tag="po")
                    ps_r = r_psum.tile([P, CHUNK], fp32, tag="pr")
                    n_kt = 4 * qc + 4
                    for kt in range(n_kt):
                        j = kt - 4 * qc  # >= 0 on diagonal tiles
                        n0 = max(0, j * P)  # trimmed col start within chunk
                        w = CHUNK - n0
                        ps_s = s_psum.tile([P, CHUNK], fp32, tag="pst")
                        nc.tensor.matmul(
                            ps_s[:, 0:w], lhsT=ktr[:, kt * P:(kt + 1) * P],
                            rhs=qtr[:, q0 + n0:q0 + CHUNK], start=True, stop=True)
                        pt = pt_pool.tile([P, CHUNK], bf16, tag="pt")
                        nc.scalar.activation(pt[:, 0:w], ps_s[:, 0:w], Exp, scale=SCALE)
                        if j >= 0:
                            # triangle mask on first P cols of the trimmed range
                            nc.vector.tensor_mul(pt[:, 0:P], pt[:, 0:P], tri_sb)
                        nc.tensor.matmul(
                            ps_o[:, n0:CHUNK],
                            lhsT=v_sb[:, kt, v_col * D:(v_col + 1) * D],
                            rhs=pt[:, 0:w], start=(kt == 0), stop=(kt == n_kt - 1))
                        nc.tensor.matmul(
                            ps_r[:, n0:CHUNK], lhsT=ones_sb, rhs=pt[:, 0:w],
                            start=(kt == 0), stop=(kt == n_kt - 1))
                    rec = rcp_pool.tile([P, CHUNK], fp32, tag="rcp")
                    nc.vector.reciprocal_approx_fast(out=rec, in_=ps_r)
                    nc.vector.tensor_mul(aot_sb[:, h, q0:q0 + CHUNK], ps_o, rec)

            # ---- phase 1 + 2, software-pipelined one head ahead ----
            v_tiles = {}
            qk_tiles = {}

            def produce(h):
                wq_sb, wk_sb = qk_w.pop(h)
                qtr = project_qk(wq_sb, "wq")
                ktr = project_qk(wk_sb, "wk")
                qk_tiles[h] = (qtr, ktr)
                if h % 2 == 0:
                    v_tiles[h // 2] = project_v(h // 2)

            produce(0)
            for h in range(HPC):
                nxt = h + 1
                if nxt < HPC:
                    load_qk_w(nxt)
                    if nxt % 2 == 0:
                        load_wv(nxt // 2)
                    produce(nxt)
                qtr, ktr = qk_tiles.pop(h)
                attention(h, qtr, ktr, v_tiles[h // 2], h % 2)

            # ---- phase 3: partial output projection y = AOT^T @ WoT ----
            for co in range(N_QC):
                wo_tiles = []
                for cb in range(HPC):  # contraction chunks == heads
                    w_sb = wo_pool.tile([P, CHUNK], bf16, tag="wo")
                    nc.sync.dma_start(
                        out=w_sb,
                        in_=wo.ap()[cb * P:(cb + 1) * P,
                                    co * CHUNK:(co + 1) * CHUNK])
                    wo_tiles.append(w_sb)
                for tt in range(N_TT):
                    ps = ph1_psum.tile([P, CHUNK], fp32, tag="ph1")
                    for cb in range(HPC):
                        nc.tensor.matmul(
                            ps, lhsT=aot_sb[:, cb, tt * P:(tt + 1) * P],
                            rhs=wo_tiles[cb],
                            start=(cb == 0), stop=(cb == HPC - 1))
                    yt = y_pool.tile([P, CHUNK], fp32, tag="y")
                    nc.scalar.copy(yt, ps)
                    nc.scalar.dma_start(
                        out=y.ap()[tt * P:(tt + 1) * P, co * CHUNK:(co + 1) * CHUNK],
                        in_=yt)

    nc.finalize()
    return nc


def _host_prep(x, Wq, Wk, Wv, Wo):
    """Build the 8 per-core input maps."""
    perm = np.concatenate([np.arange(0, D, 2), np.arange(1, D, 2)])

    inv_freq = 1.0 / ROPE_THETA ** (np.arange(0, D, 2, dtype=np.float32) / D)
    pos = np.arange(T, dtype=np.float32)
    freqs = np.einsum("i,j->ij", pos, inv_freq)  # [T, 64]
    cosT = np.cos(freqs).T.astype(np.float32)
    sinT = np.sin(freqs).T.astype(np.float32)
    cs = np.concatenate([cosT, sinT], 0).astype(BF16)
    sc = np.concatenate([sinT, cosT], 0).astype(BF16)
    tri = (np.arange(P)[:, None] <= np.arange(P)[None, :]).astype(BF16)

    halves = []
    for g in range(2):
        heads = range(g * HPC, (g + 1) * HPC)
        wq_g = np.stack([Wq[h * D:(h + 1) * D, :][perm, :].T.astype(BF16)
                         for h in heads])                       # [8, C, D]
        wk_g = np.stack([Wk[h * D:(h + 1) * D, :][perm, :].T.astype(BF16)
                         for h in heads])
        wv_g = np.stack([Wv[(g * HPC + 2 * p) * D:(g * HPC + 2 * p + 2) * D, :]
                         .T.astype(BF16) for p in range(HPC // 2)])  # [4, C, 2D]
        wo_g = Wo[:, g * HPC * D:(g + 1) * HPC * D].T.astype(BF16)   # [1024, C]
        halves.append(dict(wq=wq_g, wk=wk_g, wv=wv_g, wo=wo_g))

    in_maps = []
    for core in range(N_CORES):
        b, g = core // 2, core % 2
        m = dict(halves[g])
        m["xt"] = np.ascontiguousarray(x[b].T).astype(BF16)
        m["cs"] = cs
        m["sc"] = sc
        m["tri"] = tri
        in_maps.append(m)
    return in_maps


def kernel(x, Wq, Wk, Wv, Wo, bo):
    x = np.asarray(x, dtype=np.float32)
    Wq = np.asarray(Wq, dtype=np.float32)
    Wk = np.asarray(Wk, dtype=np.float32)
    Wv = np.asarray(Wv, dtype=np.float32)
    Wo = np.asarray(Wo, dtype=np.float32)
    bo = np.asarray(bo, dtype=np.float32)

    if "nc" not in _CACHED:
        _CACHED["nc"] = build_kernel()
    nc = _CACHED["nc"]

    in_maps = _host_prep(x, Wq, Wk, Wv, Wo)
    res = run_bass_kernel_spmd(nc, in_maps, core_ids=list(range(N_CORES)))

    out = np.empty((B, T, C), np.float32)
    for b in range(B):
        out[b] = res.results[2 * b]["y"] + res.results[2 * b + 1]["y"] + bo
    return out


# revision 12
# speedup vs baseline: 1.0617x; 1.0065x over previous
"""Trainium2 Bass kernel: multi-head attention with RoPE (causal), 8-core SPMD.

Sharding: 8 cores = 4 batches x 2 head-halves (tensor parallel over heads,
data parallel over batch). Each core computes, for its batch b and its 8
heads: QKV projections, RoPE, causal attention, and a partial output
projection. Host sums the two head-half partials per batch and adds the bias.

All TensorEngine math in bf16 with fp32 PSUM accumulation.
"""

import sys

for _p in ("/opt/trn_rl_repo",):
    if _p not in sys.path:
        sys.path.insert(0, _p)

import numpy as np
import ml_dtypes

import concourse.bass as bass
import concourse.bacc as bacc
import concourse.mybir as mybir
import concourse.tile as tile
from concourse.bass_utils import run_bass_kernel_spmd

BF16 = ml_dtypes.bfloat16

B, T, C = 4, 2048, 2048
H = 16
D = C // H  # 128
ROPE_THETA = 1000000.0
N_CORES = 8
HPC = H // 2          # heads per core (8)
P = 128               # partitions
CHUNK = 512           # moving free dim per matmul
N_CC = C // P         # 16 contraction chunks
N_QC = T // CHUNK     # 4 q-chunks
N_KT = T // P         # 16 k-tiles
N_TT = T // P         # 16 t-tiles
SCALE = 1.0 / float(np.sqrt(D))

_CACHED = {}


def build_kernel():
    """Build the SPMD Bass program (identical on all 8 cores)."""
    fp32 = mybir.dt.float32
    bf16 = mybir.dt.bfloat16

    nc = bacc.Bacc("TRN2", target_bir_lowering=False, debug=False,
                   num_devices=N_CORES)

    # Per-core DRAM inputs (bf16 unless noted)
    xt = nc.dram_tensor("xt", [C, T], bf16, kind="ExternalInput")          # x[b].T
    wq = nc.dram_tensor("wq", [HPC, C, D], bf16, kind="ExternalInput")     # Wq_h.T (perm'd)
    wk = nc.dram_tensor("wk", [HPC, C, D], bf16, kind="ExternalInput")
    wv = nc.dram_tensor("wv", [HPC // 2, C, 2 * D], bf16, kind="ExternalInput")  # head pairs
    wo = nc.dram_tensor("wo", [HPC * D, C], bf16, kind="ExternalInput")    # WoT slice
    cs = nc.dram_tensor("cs", [P, T], bf16, kind="ExternalInput")          # [cosT;sinT]
    sc = nc.dram_tensor("sc", [P, T], bf16, kind="ExternalInput")          # [sinT;cosT]
    tri = nc.dram_tensor("tri", [P, P], bf16, kind="ExternalInput")        # k<=q mask
    y = nc.dram_tensor("y", [T, C], fp32, kind="ExternalOutput")

    Exp = mybir.ActivationFunctionType.Exp

    with tile.TileContext(nc) as tc:
        with (
            tc.tile_pool(name="const", bufs=1) as const_pool,
            tc.tile_pool(name="xtp", bufs=1) as xt_pool,
            tc.tile_pool(name="wqk", bufs=4) as wqk_pool,
            tc.tile_pool(name="wvp", bufs=2) as wv_pool,
            tc.tile_pool(name="vw", bufs=4) as vw_pool,
            tc.tile_pool(name="swp", bufs=4) as swp_pool,
            tc.tile_pool(name="qktr", bufs=2) as qk_pool,
            tc.tile_pool(name="vsb", bufs=2) as v_pool,
            tc.tile_pool(name="pt", bufs=3) as pt_pool,
            tc.tile_pool(name="rcp", bufs=2) as rcp_pool,
            tc.tile_pool(name="aot", bufs=1) as aot_pool,
            tc.tile_pool(name="wop", bufs=16) as wo_pool,
            tc.tile_pool(name="yst", bufs=3) as y_pool,
            tc.tile_pool(name="ph1", bufs=3, space="PSUM") as ph1_psum,
            tc.tile_pool(name="pst", bufs=2, space="PSUM") as s_psum,
            tc.tile_pool(name="po", bufs=2, space="PSUM") as o_psum,
            tc.tile_pool(name="pr", bufs=1, space="PSUM") as r_psum,
        ):
            # ---- weight prefetch (head 0 first, before the big xt load) ----
            qk_w = {}

            def load_qk_w(h):
                wq_sb = wqk_pool.tile([P, N_CC, D], bf16, tag="wqk")
                nc.sync.dma_start(
                    out=wq_sb, in_=wq.ap()[h].rearrange("(cc p) d -> p cc d", p=P))
                wk_sb = wqk_pool.tile([P, N_CC, D], bf16, tag="wqk")
                nc.sync.dma_start(
                    out=wk_sb, in_=wk.ap()[h].rearrange("(cc p) d -> p cc d", p=P))
                qk_w[h] = (wq_sb, wk_sb)

            wv_w = {}

            def load_wv(pair):
                w_sb = wv_pool.tile([P, N_CC, 2 * D], bf16, tag="wv")
                nc.sync.dma_start(
                    out=w_sb, in_=wv.ap()[pair].rearrange("(cc p) d -> p cc d", p=P))
                wv_w[pair] = w_sb

            load_qk_w(0)

            # ---- constants / big resident tensors ----
            cs_sb = const_pool.tile([P, T], bf16)
            sc_sb = const_pool.tile([P, T], bf16)
            tri_sb = const_pool.tile([P, P], bf16)
            ones_sb = const_pool.tile([P, P], bf16)
            nc.sync.dma_start(out=cs_sb, in_=cs.ap())
            nc.sync.dma_start(out=sc_sb, in_=sc.ap())
            nc.sync.dma_start(out=tri_sb, in_=tri.ap())
            nc.vector.memset(ones_sb, 1.0)

            xt_sb = xt_pool.tile([P, N_CC, T], bf16)
            xt_r = xt.ap().rearrange("(cc p) t -> p cc t", p=P)
            for qc in range(N_QC):
                for cc in range(N_CC):
                    eng = nc.sync if cc % 2 == 0 else nc.gpsimd
                    eng.dma_start(
                        out=xt_sb[:, cc, qc * CHUNK:(qc + 1) * CHUNK],
                        in_=xt_r[:, cc, qc * CHUNK:(qc + 1) * CHUNK])
            load_wv(0)

            aot_sb = aot_pool.tile([P, HPC, T], bf16)  # attn-out^T, all heads

            def project_qk(w_sb, name):
                """Q^T (or K^T) with RoPE -> bf16 [P, T] tile."""
                out_sb = qk_pool.tile([P, T], bf16, tag=f"{name}tr")
                for qc in range(N_QC):
                    ps = ph1_psum.tile([P, CHUNK], fp32, tag="ph1")
                    for cc in range(N_CC):
                        nc.tensor.matmul(
                            ps, lhsT=w_sb[:, cc, :],
                            rhs=xt_sb[:, cc, qc * CHUNK:(qc + 1) * CHUNK],
                            start=(cc == 0), stop=(cc == N_CC - 1))
                    # RoPE: rows 0:64 = even dims (e), 64:128 = odd dims (o)
                    # re = e*cos - o*sin ; ro = o*cos + e*sin
                    v_t = vw_pool.tile([P, CHUNK], bf16, tag="vw")   # [e*cos; o*sin]
                    w_t = vw_pool.tile([P, CHUNK], bf16, tag="vw")   # [e*sin; o*cos]
                    nc.vector.tensor_mul(v_t, ps, cs_sb[:, qc * CHUNK:(qc + 1) * CHUNK])
                    nc.vector.tensor_mul(w_t, ps, sc_sb[:, qc * CHUNK:(qc + 1) * CHUNK])
                    sw = swp_pool.tile([P, CHUNK], bf16, tag="swp")
                    # sw[0:64] <- o*sin (from rows 64:128); sw[64:128] <- e*sin (rows 0:64)
                    nc.gpsimd.dma_start(out=sw[0:64, :], in_=v_t[64:128, :])
                    nc.gpsimd.dma_start(out=sw[64:128, :], in_=w_t[0:64, :])
                    sl = slice(qc * CHUNK, (qc + 1) * CHUNK)
                    nc.gpsimd.tensor_sub(out_sb[0:64, sl], v_t[0:64, :], sw[0:64, :])
                    nc.gpsimd.tensor_add(out_sb[64:128, sl], w_t[64:128, :], sw[64:128, :])
                return out_sb

            def project_v(pair):
                """V for head pair -> bf16 [P, N_KT, 2D] tile (t-partition)."""
                w_sb = wv_w.pop(pair)
                v_sb = v_pool.tile([P, N_KT, 2 * D], bf16, tag="vsb")
                for tt in range(N_TT):
                    ps = ph1_psum.tile([P, CHUNK], fp32, tag="ph1")
                    psv = ps[:, 0:2 * D]
                    for cc in range(N_CC):
                        nc.tensor.matmul(
                            psv, lhsT=xt_sb[:, cc, tt * P:(tt + 1) * P],
                            rhs=w_sb[:, cc, :],
                            start=(cc == 0), stop=(cc == N_CC - 1))
                    nc.scalar.copy(v_sb[:, tt, :], psv)
                return v_sb

            def attention(h, qtr, ktr, v_sb, v_col):
                """Causal attention for head h -> writes aot_sb[:, h, :]."""
                for qc in range(N_QC):
                    q0 = qc * CHUNK
                    ps_o = o_psum.tile([P, CHUNK], fp32, tag="po")
                    ps_r = r_psum.tile([P, CHUNK], fp32, tag="pr")
                    n_kt = 4 * qc + 4
                    for kt in range(n_kt):
                        j = kt - 4 * qc  # >= 0 on diagonal tiles
                        n0 = max(0, j * P)  # trimmed col start within chunk
                        w = CHUNK - n0
                        ps_s = s_psum.tile([P, CHUNK], fp32, tag="pst")
                        nc.tensor.matmul(
                            ps_s[:, 0:w], lhsT=ktr[:, kt * P:(kt + 1) * P],
                            rhs=qtr[:, q0 + n0:q0 + CHUNK], start=True, stop=True)
                        pt = pt_pool.tile([P, CHUNK], bf16, tag="pt")
                        nc.scalar.activation(pt[:, 0:w], ps_s[:, 0:w], Exp, scale=SCALE)
                        if j >= 0:
                            # triangle mask on first P cols of the trimmed range
                            nc.vector.tensor_mul(pt[:, 0:P], pt[:, 0:P], tri_sb)
                        nc.tensor.matmul(
                            ps_o[:, n0:CHUNK],
                            lhsT=v_sb[:, kt, v_col * D:(v_col + 1) * D],
                            rhs=pt[:, 0:w], start=(kt == 0), stop=(kt == n_kt - 1))
                        nc.tensor.matmul(
                            ps_r[:, n0:CHUNK], lhsT=ones_sb, rhs=pt[:, 0:w],
                            start=(kt == 0), stop=(kt == n_kt - 1))
                    rec = rcp_pool.tile([P, CHUNK], fp32, tag="rcp")
                    nc.vector.reciprocal_approx_fast(out=rec, in_=ps_r)
                    nc.vector.tensor_mul(aot_sb[:, h, q0:q0 + CHUNK], ps_o, rec)

            # ---- phase 1 + 2, software-pipelined one head ahead ----
            v_tiles = {}
            qk_tiles = {}

            def produce(h):
                wq_sb, wk_sb = qk_w.pop(h)
                qtr = project_qk(wq_sb, "wq")
                ktr = project_qk(wk_sb, "wk")
                qk_tiles[h] = (qtr, ktr)
                if h % 2 == 0:
                    v_tiles[h // 2] = project_v(h // 2)

            produce(0)
            for h in range(HPC):
                nxt = h + 1
                if nxt < HPC:
                    load_qk_w(nxt)
                    if nxt % 2 == 0:
                        load_wv(nxt // 2)
                    produce(nxt)
                qtr, ktr = qk_tiles.pop(h)
                attention(h, qtr, ktr, v_tiles[h // 2], h % 2)

            # ---- phase 3: partial output projection y = AOT^T @ WoT ----
            for co in range(N_QC):
                wo_tiles = []
                for cb in range(HPC):  # contraction chunks == heads
                    w_sb = wo_pool.tile([P, CHUNK], bf16, tag="wo")
                    nc.sync.dma_start(
                        out=w_sb,
                        in_=wo.ap()[cb * P:(cb + 1) * P,
                                    co * CHUNK:(co + 1) * CHUNK])
                    wo_tiles.append(w_sb)
                for tt in range(N_TT):
                    ps = ph1_psum.tile([P, CHUNK], fp32, tag="ph1")
                    for cb in range(HPC):
                        nc.tensor.matmul(
                            ps, lhsT=aot_sb[:, cb, tt * P:(tt + 1) * P],
                            rhs=wo_tiles[cb],
                            start=(cb == 0), stop=(cb == HPC - 1))
                    yt = y_pool.tile([P, CHUNK], fp32, tag="y")
                    nc.scalar.copy(yt, ps)
                    nc.scalar.dma_start(
                        out=y.ap()[tt * P:(tt + 1) * P, co * CHUNK:(co + 1) * CHUNK],
                        in_=yt)

    nc.finalize()
    return nc


def _host_prep(x, Wq, Wk, Wv, Wo):
    """Build the 8 per-core input maps."""
    perm = np.concatenate([np.arange(0, D, 2), np.arange(1, D, 2)])

    inv_freq = 1.0 / ROPE_THETA ** (np.arange(0, D, 2, dtype=np.float32) / D)
    pos = np.arange(T, dtype=np.float32)
    freqs = np.einsum("i,j->ij", pos, inv_freq)  # [T, 64]
    cosT = np.cos(freqs).T.astype(np.float32)
    sinT = np.sin(freqs).T.astype(np.float32)
    cs = np.concatenate([cosT, sinT], 0).astype(BF16)
    sc = np.concatenate([sinT, cosT], 0).astype(BF16)
    tri = (np.arange(P)[:, None] <= np.arange(P)[None, :]).astype(BF16)

    halves = []
    for g in range(2):
        heads = range(g * HPC, (g + 1) * HPC)
        wq_g = np.stack([Wq[h * D:(h + 1) * D, :][perm, :].T.astype(BF16)
                         for h in heads])                       # [8, C, D]
        wk_g = np.stack([Wk[h * D:(h + 1) * D, :][perm, :].T.astype(BF16)
                         for h in heads])
        wv_g = np.stack([Wv[(g * HPC + 2 * p) * D:(g * HPC + 2 * p + 2) * D, :]
                         .T.astype(BF16) for p in range(HPC // 2)])  # [4, C, 2D]
        wo_g = Wo[:, g * HPC * D:(g + 1) * HPC * D].T.astype(BF16)   # [1024, C]
        halves.append(dict(wq=wq_g, wk=wk_g, wv=wv_g, wo=wo_g))

    in_maps = []
    for core in range(N_CORES):
        b, g = core // 2, core % 2
        m = dict(halves[g])
        m["xt"] = np.ascontiguousarray(x[b].T).astype(BF16)
        m["cs"] = cs
        m["sc"] = sc
        m["tri"] = tri
        in_maps.append(m)
    return in_maps


def kernel(x, Wq, Wk, Wv, Wo, bo):
    x = np.asarray(x, dtype=np.float32)
    Wq = np.asarray(Wq, dtype=np.float32)
    Wk = np.asarray(Wk, dtype=np.float32)
    Wv = np.asarray(Wv, dtype=np.float32)
    Wo = np.asarray(Wo, dtype=np.float32)
    bo = np.asarray(bo, dtype=np.float32)

    if "nc" not in _CACHED:
        _CACHED["nc"] = build_kernel()
    nc = _CACHED["nc"]

    in_maps = _host_prep(x, Wq, Wk, Wv, Wo)
    res = run_bass_kernel_spmd(nc, in_maps, core_ids=list(range(N_CORES)))

    out = np.empty((B, T, C), np.float32)
    for b in range(B):
        out[b] = res.results[2 * b]["y"] + res.results[2 * b + 1]["y"] + bo
    return out
